# revision 15
# baseline (speedup 1.0000x reference)
"""Trainium2 Bass kernel: sliding-window multi-head attention with ALiBi.

Reference computation (B=2, S=4096, E=512, H=8, D=64, window 513):
    q = (inputs_q @ w_q);  k = (inputs_kv @ w_k);  v = (inputs_kv @ w_v)
    att = softmax(q k^T / 8 + alibi, sliding window +-256)
    out = (att v) @ w_o

Sharding: 8 cores = 2 batches x 4 sequence quarters (1024 q rows per core).
Each core gets its kv slice with a 256-row halo (zero-padded at sequence
edges).  All cores run the identical program (SPMD); edge handling is pure
data:
  - zero-padded X_kv makes K=V=0 on out-of-range rows,
  - a host-provided validity column appended to V makes the softmax
    denominator (accumulated by the same AV matmul) skip those rows,
  - the window/ALiBi mask is applied as a multiplicative exp-mask after
    exp(): P = exp(S) * G, where G[h] is Toeplitz in (kv - q) and therefore
    shared by all query blocks (G = exp(-slope_h * |rel|) * [|rel| <= 256]);
    the host pre-unrolls it into the 6-chunk score layout.

Layout: scores are computed transposed (S^T[kv, q]) so the AV matmul needs
no on-chip transposes: lhsT = [V | valid], rhs = P^T gives O^T[d, q] plus
the denominator row.  All 6 kv-chunks of a q-block live in one [128,1536]
PSUM tile (3 banks; the even chunk of each bank carries start=True, which
zeroes the whole 2KB zero-region, so the odd chunk runs start=False), so
exp() and the mask multiply are single big ops.  Normalization: reciprocal
of the denominator row is partition-broadcast on GPSIMD (its only op - no
gpsimd library switching) and multiplied in on DVE.  The final projection
consumes O^T tiles directly as stationary operands.  All matmul operands
are float32r (full PE rate at moving dim >= 256).
"""

import os
import sys

if "/opt/trn_rl_repo" not in sys.path:
    sys.path.insert(0, "/opt/trn_rl_repo")

import numpy as np

import concourse.bacc as bacc
import concourse.mybir as mybir
import concourse.tile as tile
from concourse.bass_utils import run_bass_kernel_spmd

# ---------------------------------------------------------------- geometry
B, S, E = 2, 4096, 512
H, D = 8, 64
HD = H * D              # 512
HALF = 256              # window half-width (ATTENTION_WINDOW=512 -> 513 wide)
NCORES = 8
SQ = 4                  # sequence shards per batch
QROWS = S // SQ         # 1024 q rows per core
KVROWS = QROWS + 2 * HALF   # 1536 kv rows per core (with halo)
QB = 4                  # q blocks per core
QBLK = QROWS // QB      # 256 q cols per block
NCH = 6                 # kv chunks per q block
CBLK = 128              # kv chunk rows
SP6 = NCH * QBLK        # 1536: all chunks of a q block side by side

F32 = mybir.dt.float32
F32R = mybir.dt.float32r
BF16 = mybir.dt.bfloat16
PV_DT = F32R            # dtype of exp/mask/P^T/V path (F32R or BF16)

PWORK_BUFS = int(os.environ.get("K_PWORK", "3"))
GSQRT = os.environ.get("K_GSQRT", "0") == "1"   # derive G_h by chained sqrt

_CACHE = {}


def _build_program(repeats=1):
    """Build + compile the SPMD program (cached per process).

    repeats > 1 re-runs the whole computation that many times (same inputs,
    same outputs) - used only for wall-clock HW timing by difference.
    """
    key = ("nc", repeats)
    if key in _CACHE:
        return _CACHE[key]

    nc = bacc.Bacc("TRN2", target_bir_lowering=False, debug=False,
                   enable_asserts=True)

    xq_d = nc.dram_tensor("xqT", [E, QROWS], F32R, kind="ExternalInput")
    xkv_d = nc.dram_tensor("xkvT", [E, KVROWS], F32R, kind="ExternalInput")
    wq_d = nc.dram_tensor("wq", [E, HD], F32R, kind="ExternalInput")
    wk_d = nc.dram_tensor("wk", [E, HD], F32R, kind="ExternalInput")
    wv_d = nc.dram_tensor("wv", [E, HD], F32R, kind="ExternalInput")
    wo_d = nc.dram_tensor("wo", [HD, E], F32R, kind="ExternalInput")
    g_d = nc.dram_tensor("gmask", [1 if GSQRT else H, 128, SP6],
                     PV_DT, kind="ExternalInput")
    val_d = nc.dram_tensor("validc", [128, KVROWS // CBLK], F32,
                           kind="ExternalInput")
    y_d = nc.dram_tensor("y", [QROWS, E], F32, kind="ExternalOutput")

    EXP = mybir.ActivationFunctionType.Exp

    with tile.TileContext(nc) as tc:
        with (
            tc.tile_pool(name="wts", bufs=1) as wts,
            tc.tile_pool(name="bigx", bufs=8) as bigx,
            tc.tile_pool(name="proj", bufs=1) as proj,
            tc.tile_pool(name="pwork", bufs=PWORK_BUFS) as pwork,
            tc.tile_pool(name="small", bufs=4) as small,
            tc.tile_pool(name="bigp", bufs=2, space="PSUM") as bigp,
            tc.tile_pool(name="otp", bufs=2, space="PSUM") as otp,
        ):
            # ---- load weights (E on partitions, 4 tiles each)
            def load4(dram, name, cols):
                ts = []
                for e in range(4):
                    t = wts.tile([128, cols], F32R, tag=f"{name}{e}",
                                 name=f"{name}{e}")
                    nc.sync.dma_start(t[:], dram.ap()[128 * e:128 * (e + 1), :])
                    ts.append(t)
                return ts

            wq_sb = load4(wq_d, "wq", HD)
            wk_sb = load4(wk_d, "wk", HD)
            wv_sb = load4(wv_d, "wv", HD)
            wo_sb = load4(wo_d, "wo", E)

            valid_sb = small.tile([128, KVROWS // CBLK], F32, tag="validc",
                                  name="validc")
            nc.sync.dma_start(valid_sb[:], val_d.ap()[:])
            ones8 = small.tile([128, H], F32, tag="ones8", name="ones8")
            nc.vector.memset(ones8[:], 1.0)

            # ---- persistent activation tiles
            qt_sb = [proj.tile([128, QROWS], F32R, tag=f"qt{t}", name=f"qt{t}")
                     for t in range(4)]
            kt_sb = [proj.tile([128, KVROWS], F32R, tag=f"kt{t}",
                               name=f"kt{t}") for t in range(4)]
            # V tiles: head h occupies cols [65h, 65h+64), col 65h+64 = valid
            v_sb = [proj.tile([128, 65 * H], PV_DT, tag=f"v{b}", name=f"v{b}")
                    for b in range(KVROWS // CBLK)]
            ot_sb = [proj.tile([128, QROWS], F32R, tag=f"ot{t}", name=f"ot{t}")
                     for t in range(4)]

            for rep in range(repeats):
                xq_sb = []
                for e in range(4):
                    t = bigx.tile([128, KVROWS], F32R, tag="bigx",
                                  name=f"xq{e}")
                    nc.sync.dma_start(t[:, :QROWS],
                                      xq_d.ap()[128 * e:128 * (e + 1), :])
                    xq_sb.append(t)
                xkv_sb = []
                for e in range(4):
                    t = bigx.tile([128, KVROWS], F32R, tag="bigx",
                                  name=f"xkv{e}")
                    nc.sync.dma_start(t[:],
                                      xkv_d.ap()[128 * e:128 * (e + 1), :])
                    xkv_sb.append(t)

                # ---- projections
                for t in range(4):
                    for n in range(QROWS // 512):
                        ps = bigp.tile([128, SP6], F32, tag="bigp",
                                       name="psq")
                        for e in range(4):
                            nc.tensor.matmul(
                                ps[:, :512],
                                lhsT=wq_sb[e][:, 128 * t:128 * (t + 1)],
                                rhs=xq_sb[e][:, 512 * n:512 * (n + 1)],
                                start=(e == 0), stop=(e == 3))
                        nc.scalar.copy(qt_sb[t][:, 512 * n:512 * (n + 1)],
                                       ps[:, :512])

                for t in range(4):
                    for n in range(KVROWS // 512):
                        ps = bigp.tile([128, SP6], F32, tag="bigp",
                                       name="psk")
                        for e in range(4):
                            nc.tensor.matmul(
                                ps[:, :512],
                                lhsT=wk_sb[e][:, 128 * t:128 * (t + 1)],
                                rhs=xkv_sb[e][:, 512 * n:512 * (n + 1)],
                                start=(e == 0), stop=(e == 3))
                        nc.scalar.copy(kt_sb[t][:, 512 * n:512 * (n + 1)],
                                       ps[:, :512])

                for blk in range(KVROWS // CBLK):
                    ps = bigp.tile([128, SP6], F32, tag="bigp", name="psv")
                    for e in range(4):
                        nc.tensor.matmul(
                            ps[:, :512],
                            lhsT=xkv_sb[e][:, 128 * blk:128 * (blk + 1)],
                            rhs=wv_sb[e][:],
                            start=(e == 0), stop=(e == 3))
                    vv = v_sb[blk][:].rearrange("p (h c) -> p h c", c=65)
                    nc.vector.tensor_copy(
                        vv[:, :, 0:64],
                        ps[:, :512].rearrange("p (h c) -> p h c", c=64))
                    nc.vector.tensor_scalar_mul(
                        vv[:, :, 64], ones8[:],
                        valid_sb[:, blk:blk + 1])

                # ---- Toeplitz exp-masks (reuse bigx slots once proj done)
                g_sb = [None] * H
                if GSQRT:
                    # Ship only G_{H-1} (slope 2^-8) and derive the rest by
                    # squaring: G_{h} = G_{h+1}^2 (slopes double downward).
                    # Squaring underflows to 0 only where the true mask is
                    # negligibly small; Square lives in every ACT table.
                    for h in range(H - 1, -1, -1):
                        t = bigx.tile([128, KVROWS], PV_DT, tag="bigx",
                                      name=f"g{h}")
                        if h == H - 1:
                            nc.sync.dma_start(t[:, :SP6], g_d.ap()[0])
                        else:
                            nc.scalar.square(t[:, :SP6],
                                             g_sb[h + 1][:, :SP6])
                        g_sb[h] = t
                else:
                    for h in range(H):
                        t = bigx.tile([128, KVROWS], PV_DT, tag="bigx",
                                      name=f"g{h}")
                        nc.sync.dma_start(t[:, :SP6], g_d.ap()[h])
                        g_sb[h] = t

                # ---- attention (descending h: with GSQRT the deepest
                # chain element g0 is needed last)
                for h in range(H - 1, -1, -1):
                    for qb in range(QB):
                        th, ph = h // 2, 64 * (h % 2)
                        qs = qt_sb[th][ph:ph + 64, QBLK * qb:QBLK * (qb + 1)]
                        sp = bigp.tile([128, SP6], F32, tag="bigp", name="sp6")
                        for c in range(NCH):
                            k0 = QBLK * qb + CBLK * c
                            nc.tensor.matmul(
                                sp[:, QBLK * c:QBLK * (c + 1)],
                                lhsT=kt_sb[th][ph:ph + 64, k0:k0 + CBLK],
                                rhs=qs,
                                start=(c % 2 == 0), stop=True,
                                skip_group_check=(c % 2 == 1))
                        pe6 = pwork.tile([128, SP6], PV_DT, tag="pw",
                                         name="pe6")
                        nc.scalar.activation(pe6[:], sp[:], EXP)
                        pm6 = pwork.tile([128, SP6], PV_DT, tag="pw",
                                         name="pm6")
                        nc.vector.tensor_mul(pm6[:], pe6[:], g_sb[h][:, :SP6])
                        ot = otp.tile([65, QBLK], F32, tag="otp", name="ot")
                        for c in range(NCH):
                            nc.tensor.matmul(
                                ot[:],
                                lhsT=v_sb[2 * qb + c][:, 65 * h:65 * h + 65],
                                rhs=pm6[:, QBLK * c:QBLK * (c + 1)],
                                start=(c == 0), stop=(c == NCH - 1))
                        rec = small.tile([1, QBLK], F32, tag="rec", name="rec")
                        nc.vector.reciprocal(rec[:], ot[64:65, :])
                        bc = pwork.tile([64, QBLK], F32, tag="bc", name="bc")
                        nc.gpsimd.partition_broadcast(bc[:], rec[:])
                        nc.vector.tensor_mul(
                            ot_sb[th][ph:ph + 64, QBLK * qb:QBLK * (qb + 1)],
                            ot[0:64, :], bc[:])

                # ---- output projection
                for yb in range(QROWS // 128):
                    yp = bigp.tile([128, SP6], F32, tag="bigp", name="yp")
                    for t in range(4):
                        nc.tensor.matmul(
                            yp[:, :512],
                            lhsT=ot_sb[t][:, 128 * yb:128 * (yb + 1)],
                            rhs=wo_sb[t][:],
                            start=(t == 0), stop=(t == 3))
                    ystage = pwork.tile([128, 512], F32, tag="ys",
                                        name="ystage")
                    nc.scalar.copy(ystage[:], yp[:, :512])
                    nc.sync.dma_start(y_d.ap()[128 * yb:128 * (yb + 1), :],
                                      ystage[:])

    nc.compile()
    _CACHE[key] = nc
    return nc


def build_in_maps(inputs_q, inputs_kv, w_q, w_k, w_v, w_o):
    """Host-side sharding: slice/transpose/pad per core + mask tensors."""
    inputs_q = np.asarray(inputs_q, np.float32)
    inputs_kv = np.asarray(inputs_kv, np.float32)
    np_pv = mybir.dt.np(PV_DT)

    wq = np.ascontiguousarray(np.asarray(w_q, np.float32) * 0.125)
    wk = np.ascontiguousarray(np.asarray(w_k, np.float32))
    wv = np.ascontiguousarray(np.asarray(w_v, np.float32))
    wo = np.ascontiguousarray(np.asarray(w_o, np.float32))

    # Toeplitz exp-mask, pre-unrolled into the 6-chunk score layout:
    # chunk c, kv row r, q col i -> rel = i - r - 128c + 256
    slopes = np.array([2.0 ** (-(i + 1)) for i in range(H)], np.float64)
    r = np.arange(128)[:, None]
    i = np.arange(QBLK)[None, :]
    nheads_g = 1 if GSQRT else H
    g = np.empty((nheads_g, 128, SP6), np_pv)
    for c in range(NCH):
        rel = i - r - 128 * c + 256
        band = (np.abs(rel) <= HALF)
        for h in range(nheads_g):
            s = slopes[H - 1] if GSQRT else slopes[h]
            g[h, :, QBLK * c:QBLK * (c + 1)] = (
                np.exp(-s * np.abs(rel)) * band).astype(np_pv)

    in_maps = []
    for c in range(NCORES):
        b, sq = divmod(c, SQ)
        g0 = QROWS * sq
        xq = np.ascontiguousarray(inputs_q[b, g0:g0 + QROWS, :].T)
        kvlo = g0 - HALF
        lo, hi = max(0, kvlo), min(S, g0 + QROWS + HALF)
        xkv = np.zeros((E, KVROWS), np.float32)
        xkv[:, lo - kvlo:hi - kvlo] = inputs_kv[b, lo:hi, :].T
        valid = np.zeros((KVROWS,), np.float32)
        valid[lo - kvlo:hi - kvlo] = 1.0
        validc = np.ascontiguousarray(valid.reshape(KVROWS // CBLK, CBLK).T)
        in_maps.append({
            "xqT": xq, "xkvT": xkv,
            "wq": wq, "wk": wk, "wv": wv, "wo": wo,
            "gmask": g, "validc": validc,
        })
    return in_maps


def assemble_output(results):
    out = np.empty((B, S, E), np.float32)
    for c in range(NCORES):
        b, sq = divmod(c, SQ)
        out[b, QROWS * sq:QROWS * (sq + 1), :] = results[c]["y"]
    return out


def kernel(inputs_q, inputs_kv, w_q, w_k, w_v, w_o):
    nc = _build_program()
    in_maps = build_in_maps(inputs_q, inputs_kv, w_q, w_k, w_v, w_o)
    res = run_bass_kernel_spmd(nc, in_maps, core_ids=list(range(NCORES)))
    return assemble_output(res.results)


# revision 19
# speedup vs baseline: 1.0250x; 1.0250x over previous
"""Trainium2 Bass kernel: sliding-window multi-head attention with ALiBi.

Reference computation (B=2, S=4096, E=512, H=8, D=64, window 513):
    q = (inputs_q @ w_q);  k = (inputs_kv @ w_k);  v = (inputs_kv @ w_v)
    att = softmax(q k^T / 8 + alibi, sliding window +-256)
    out = (att v) @ w_o

Sharding: 8 cores = 2 batches x 4 sequence quarters (1024 q rows per core).
Each core gets its kv slice with a 256-row halo (zero-padded at sequence
edges).  All cores run the identical program (SPMD); edge handling is pure
data:
  - zero-padded X_kv makes K=V=0 on out-of-range rows,
  - a host-provided validity column appended to V makes the softmax
    denominator (accumulated by the same AV matmul) skip those rows,
  - the window/ALiBi mask is applied as a multiplicative exp-mask after
    exp(): P = exp(S) * G, where G[h] is Toeplitz in (kv - q) and therefore
    shared by all query blocks (G = exp(-slope_h * |rel|) * [|rel| <= 256]);
    the host pre-unrolls it into the 6-chunk score layout.

Layout: scores are computed transposed (S^T[kv, q]) so the AV matmul needs
no on-chip transposes: lhsT = [V | valid], rhs = P^T gives O^T[d, q] plus
the denominator row.  All 6 kv-chunks of a q-block live in one [128,1536]
PSUM tile (3 banks; the even chunk of each bank carries start=True, which
zeroes the whole 2KB zero-region, so the odd chunk runs start=False), so
exp() and the mask multiply are single big ops.  Normalization: reciprocal
of the denominator row is partition-broadcast on GPSIMD (its only op - no
gpsimd library switching) and multiplied in on DVE.  The final projection
consumes O^T tiles directly as stationary operands.  All matmul operands
are float32r (full PE rate at moving dim >= 256).
"""

import os
import sys

if "/opt/trn_rl_repo" not in sys.path:
    sys.path.insert(0, "/opt/trn_rl_repo")

import numpy as np

import concourse.bacc as bacc
import concourse.mybir as mybir
import concourse.tile as tile
from concourse.bass_utils import run_bass_kernel_spmd

# ---------------------------------------------------------------- geometry
B, S, E = 2, 4096, 512
H, D = 8, 64
HD = H * D              # 512
HALF = 256              # window half-width (ATTENTION_WINDOW=512 -> 513 wide)
NCORES = 8
SQ = 4                  # sequence shards per batch
QROWS = S // SQ         # 1024 q rows per core
KVROWS = QROWS + 2 * HALF   # 1536 kv rows per core (with halo)
QB = 4                  # q blocks per core
QBLK = QROWS // QB      # 256 q cols per block
NCH = 6                 # kv chunks per q block
CBLK = 128              # kv chunk rows
SP6 = NCH * QBLK        # 1536: all chunks of a q block side by side

F32 = mybir.dt.float32
F32R = mybir.dt.float32r
BF16 = mybir.dt.bfloat16
PV_DT = F32R            # dtype of exp/mask/P^T/V path (F32R or BF16)

PWORK_BUFS = int(os.environ.get("K_PWORK", "4"))
GSQRT = os.environ.get("K_GSQRT", "0") == "1"   # derive G_h by chained sqrt
VCOPY_ACT = os.environ.get("K_VCOPY", "act") == "act"
QKCOPY_DVE = os.environ.get("K_QKCOPY", "act") == "dve"

_CACHE = {}


def _build_program(repeats=1):
    """Build + compile the SPMD program (cached per process).

    repeats > 1 re-runs the whole computation that many times (same inputs,
    same outputs) - used only for wall-clock HW timing by difference.
    """
    key = ("nc", repeats)
    if key in _CACHE:
        return _CACHE[key]

    nc = bacc.Bacc("TRN2", target_bir_lowering=False, debug=False,
                   enable_asserts=True)

    xq_d = nc.dram_tensor("xqT", [E, QROWS], F32R, kind="ExternalInput")
    xkv_d = nc.dram_tensor("xkvT", [E, KVROWS], F32R, kind="ExternalInput")
    wq_d = nc.dram_tensor("wq", [E, HD], F32R, kind="ExternalInput")
    wk_d = nc.dram_tensor("wk", [E, HD], F32R, kind="ExternalInput")
    wv_d = nc.dram_tensor("wv", [E, HD], F32R, kind="ExternalInput")
    wo_d = nc.dram_tensor("wo", [HD, E], F32R, kind="ExternalInput")
    g_d = nc.dram_tensor("gmask", [1 if GSQRT else H, 128, SP6],
                     PV_DT, kind="ExternalInput")
    val_d = nc.dram_tensor("validc", [128, KVROWS // CBLK], F32,
                           kind="ExternalInput")
    y_d = nc.dram_tensor("y", [QROWS, E], F32, kind="ExternalOutput")

    EXP = mybir.ActivationFunctionType.Exp

    with tile.TileContext(nc) as tc:
        with (
            tc.tile_pool(name="wts", bufs=1) as wts,
            tc.tile_pool(name="bigx", bufs=8) as bigx,
            tc.tile_pool(name="proj", bufs=1) as proj,
            tc.tile_pool(name="pwork", bufs=PWORK_BUFS) as pwork,
            tc.tile_pool(name="small", bufs=4) as small,
            tc.tile_pool(name="bigp", bufs=2, space="PSUM") as bigp,
            tc.tile_pool(name="otp", bufs=2, space="PSUM") as otp,
        ):
            # ---- load weights (E on partitions, 4 tiles each)
            def load4(dram, name, cols):
                ts = []
                for e in range(4):
                    t = wts.tile([128, cols], F32R, tag=f"{name}{e}",
                                 name=f"{name}{e}")
                    nc.sync.dma_start(t[:], dram.ap()[128 * e:128 * (e + 1), :])
                    ts.append(t)
                return ts

            wq_sb = load4(wq_d, "wq", HD)
            wk_sb = load4(wk_d, "wk", HD)
            wv_sb = load4(wv_d, "wv", HD)
            wo_sb = load4(wo_d, "wo", E)

            valid_sb = small.tile([128, KVROWS // CBLK], F32, tag="validc",
                                  name="validc")
            nc.sync.dma_start(valid_sb[:], val_d.ap()[:])
            ones8 = small.tile([128, H], F32, tag="ones8", name="ones8")
            nc.vector.memset(ones8[:], 1.0)

            # ---- persistent activation tiles
            qt_sb = [proj.tile([128, QROWS], F32R, tag=f"qt{t}", name=f"qt{t}")
                     for t in range(4)]
            kt_sb = [proj.tile([128, KVROWS], F32R, tag=f"kt{t}",
                               name=f"kt{t}") for t in range(4)]
            # V tiles: head h occupies cols [65h, 65h+64), col 65h+64 = valid
            v_sb = [proj.tile([128, 65 * H], PV_DT, tag=f"v{b}", name=f"v{b}")
                    for b in range(KVROWS // CBLK)]
            ot_sb = [proj.tile([128, QROWS], F32R, tag=f"ot{t}", name=f"ot{t}")
                     for t in range(4)]

            for rep in range(repeats):
                xq_sb = []
                for e in range(4):
                    t = bigx.tile([128, KVROWS], F32R, tag="bigx",
                                  name=f"xq{e}")
                    nc.sync.dma_start(t[:, :QROWS],
                                      xq_d.ap()[128 * e:128 * (e + 1), :])
                    xq_sb.append(t)
                xkv_sb = []
                for e in range(4):
                    t = bigx.tile([128, KVROWS], F32R, tag="bigx",
                                  name=f"xkv{e}")
                    nc.sync.dma_start(t[:],
                                      xkv_d.ap()[128 * e:128 * (e + 1), :])
                    xkv_sb.append(t)

                # ---- projections
                for t in range(4):
                    for n in range(QROWS // 512):
                        ps = bigp.tile([128, SP6], F32, tag="bigp",
                                       name="psq")
                        for e in range(4):
                            nc.tensor.matmul(
                                ps[:, :512],
                                lhsT=wq_sb[e][:, 128 * t:128 * (t + 1)],
                                rhs=xq_sb[e][:, 512 * n:512 * (n + 1)],
                                start=(e == 0), stop=(e == 3))
                        if QKCOPY_DVE:
                            nc.vector.tensor_copy(
                                qt_sb[t][:, 512 * n:512 * (n + 1)],
                                ps[:, :512])
                        else:
                            nc.scalar.copy(
                                qt_sb[t][:, 512 * n:512 * (n + 1)],
                                ps[:, :512])

                for t in range(4):
                    for n in range(KVROWS // 512):
                        ps = bigp.tile([128, SP6], F32, tag="bigp",
                                       name="psk")
                        for e in range(4):
                            nc.tensor.matmul(
                                ps[:, :512],
                                lhsT=wk_sb[e][:, 128 * t:128 * (t + 1)],
                                rhs=xkv_sb[e][:, 512 * n:512 * (n + 1)],
                                start=(e == 0), stop=(e == 3))
                        if QKCOPY_DVE:
                            nc.vector.tensor_copy(
                                kt_sb[t][:, 512 * n:512 * (n + 1)],
                                ps[:, :512])
                        else:
                            nc.scalar.copy(
                                kt_sb[t][:, 512 * n:512 * (n + 1)],
                                ps[:, :512])

                for blk in range(KVROWS // CBLK):
                    ps = bigp.tile([128, SP6], F32, tag="bigp", name="psv")
                    for e in range(4):
                        nc.tensor.matmul(
                            ps[:, :512],
                            lhsT=xkv_sb[e][:, 128 * blk:128 * (blk + 1)],
                            rhs=wv_sb[e][:],
                            start=(e == 0), stop=(e == 3))
                    vv = v_sb[blk][:].rearrange("p (h c) -> p h c", c=65)
                    veng = nc.scalar if VCOPY_ACT else nc.vector
                    (veng.copy if VCOPY_ACT else nc.vector.tensor_copy)(
                        vv[:, :, 0:64],
                        ps[:, :512].rearrange("p (h c) -> p h c", c=64))
                    nc.vector.tensor_scalar_mul(
                        vv[:, :, 64], ones8[:],
                        valid_sb[:, blk:blk + 1])

                # ---- Toeplitz exp-masks (reuse bigx slots once proj done)
                g_sb = [None] * H
                if GSQRT:
                    # Ship only G_{H-1} (slope 2^-8) and derive the rest by
                    # squaring: G_{h} = G_{h+1}^2 (slopes double downward).
                    # Squaring underflows to 0 only where the true mask is
                    # negligibly small; Square lives in every ACT table.
                    for h in range(H - 1, -1, -1):
                        t = bigx.tile([128, KVROWS], PV_DT, tag="bigx",
                                      name=f"g{h}")
                        if h == H - 1:
                            nc.sync.dma_start(t[:, :SP6], g_d.ap()[0])
                        else:
                            nc.scalar.square(t[:, :SP6],
                                             g_sb[h + 1][:, :SP6])
                        g_sb[h] = t
                else:
                    for h in range(H):
                        t = bigx.tile([128, KVROWS], PV_DT, tag="bigx",
                                      name=f"g{h}")
                        nc.sync.dma_start(t[:, :SP6], g_d.ap()[h])
                        g_sb[h] = t

                # ---- attention (descending h: with GSQRT the deepest
                # chain element g0 is needed last)
                for h in range(H - 1, -1, -1):
                    for qb in range(QB):
                        th, ph = h // 2, 64 * (h % 2)
                        qs = qt_sb[th][ph:ph + 64, QBLK * qb:QBLK * (qb + 1)]
                        sp = bigp.tile([128, SP6], F32, tag="bigp", name="sp6")
                        for c in range(NCH):
                            k0 = QBLK * qb + CBLK * c
                            nc.tensor.matmul(
                                sp[:, QBLK * c:QBLK * (c + 1)],
                                lhsT=kt_sb[th][ph:ph + 64, k0:k0 + CBLK],
                                rhs=qs,
                                start=(c % 2 == 0), stop=True,
                                skip_group_check=(c % 2 == 1))
                        pe6 = pwork.tile([128, SP6], PV_DT, tag="pw",
                                         name="pe6")
                        nc.scalar.activation(pe6[:], sp[:], EXP)
                        pm6 = pwork.tile([128, SP6], PV_DT, tag="pw",
                                         name="pm6")
                        nc.vector.tensor_mul(pm6[:], pe6[:], g_sb[h][:, :SP6])
                        ot = otp.tile([65, QBLK], F32, tag="otp", name="ot")
                        for c in range(NCH):
                            nc.tensor.matmul(
                                ot[:],
                                lhsT=v_sb[2 * qb + c][:, 65 * h:65 * h + 65],
                                rhs=pm6[:, QBLK * c:QBLK * (c + 1)],
                                start=(c == 0), stop=(c == NCH - 1))
                        rec = small.tile([1, QBLK], F32, tag="rec", name="rec")
                        nc.vector.reciprocal(rec[:], ot[64:65, :])
                        bc = pwork.tile([64, QBLK], F32, tag="bc", name="bc")
                        nc.gpsimd.partition_broadcast(bc[:], rec[:])
                        nc.vector.tensor_mul(
                            ot_sb[th][ph:ph + 64, QBLK * qb:QBLK * (qb + 1)],
                            ot[0:64, :], bc[:])

                # ---- output projection
                for yb in range(QROWS // 128):
                    yp = bigp.tile([128, SP6], F32, tag="bigp", name="yp")
                    for t in range(4):
                        nc.tensor.matmul(
                            yp[:, :512],
                            lhsT=ot_sb[t][:, 128 * yb:128 * (yb + 1)],
                            rhs=wo_sb[t][:],
                            start=(t == 0), stop=(t == 3))
                    ystage = pwork.tile([128, 512], F32, tag="ys",
                                        name="ystage")
                    nc.scalar.copy(ystage[:], yp[:, :512])
                    nc.sync.dma_start(y_d.ap()[128 * yb:128 * (yb + 1), :],
                                      ystage[:])

    nc.compile()
    _CACHE[key] = nc
    return nc


def build_in_maps(inputs_q, inputs_kv, w_q, w_k, w_v, w_o):
    """Host-side sharding: slice/transpose/pad per core + mask tensors."""
    inputs_q = np.asarray(inputs_q, np.float32)
    inputs_kv = np.asarray(inputs_kv, np.float32)
    np_pv = mybir.dt.np(PV_DT)

    wq = np.ascontiguousarray(np.asarray(w_q, np.float32) * 0.125)
    wk = np.ascontiguousarray(np.asarray(w_k, np.float32))
    wv = np.ascontiguousarray(np.asarray(w_v, np.float32))
    wo = np.ascontiguousarray(np.asarray(w_o, np.float32))

    # Toeplitz exp-mask, pre-unrolled into the 6-chunk score layout:
    # chunk c, kv row r, q col i -> rel = i - r - 128c + 256
    slopes = np.array([2.0 ** (-(i + 1)) for i in range(H)], np.float64)
    r = np.arange(128)[:, None]
    i = np.arange(QBLK)[None, :]
    nheads_g = 1 if GSQRT else H
    g = np.empty((nheads_g, 128, SP6), np_pv)
    for c in range(NCH):
        rel = i - r - 128 * c + 256
        band = (np.abs(rel) <= HALF)
        for h in range(nheads_g):
            s = slopes[H - 1] if GSQRT else slopes[h]
            g[h, :, QBLK * c:QBLK * (c + 1)] = (
                np.exp(-s * np.abs(rel)) * band).astype(np_pv)

    in_maps = []
    for c in range(NCORES):
        b, sq = divmod(c, SQ)
        g0 = QROWS * sq
        xq = np.ascontiguousarray(inputs_q[b, g0:g0 + QROWS, :].T)
        kvlo = g0 - HALF
        lo, hi = max(0, kvlo), min(S, g0 + QROWS + HALF)
        xkv = np.zeros((E, KVROWS), np.float32)
        xkv[:, lo - kvlo:hi - kvlo] = inputs_kv[b, lo:hi, :].T
        valid = np.zeros((KVROWS,), np.float32)
        valid[lo - kvlo:hi - kvlo] = 1.0
        validc = np.ascontiguousarray(valid.reshape(KVROWS // CBLK, CBLK).T)
        in_maps.append({
            "xqT": xq, "xkvT": xkv,
            "wq": wq, "wk": wk, "wv": wv, "wo": wo,
            "gmask": g, "validc": validc,
        })
    return in_maps


def assemble_output(results):
    out = np.empty((B, S, E), np.float32)
    for c in range(NCORES):
        b, sq = divmod(c, SQ)
        out[b, QROWS * sq:QROWS * (sq + 1), :] = results[c]["y"]
    return out


def kernel(inputs_q, inputs_kv, w_q, w_k, w_v, w_o):
    nc = _build_program()
    in_maps = build_in_maps(inputs_q, inputs_kv, w_q, w_k, w_v, w_o)
    res = run_bass_kernel_spmd(nc, in_maps, core_ids=list(range(NCORES)))
    return assemble_output(res.results)


# revision 23
# speedup vs baseline: 1.2165x; 1.1868x over previous
"""Trainium2 Bass kernel: sliding-window multi-head attention with ALiBi.

Reference computation (B=2, S=4096, E=512, H=8, D=64, window 513):
    q = (inputs_q @ w_q);  k = (inputs_kv @ w_k);  v = (inputs_kv @ w_v)
    att = softmax(q k^T / 8 + alibi, sliding window +-256)
    out = (att v) @ w_o

Sharding: 8 cores = 2 batches x 4 sequence quarters (1024 q rows per core).
Each core gets its kv slice with a 256-row halo (zero-padded at sequence
edges).  All cores run the identical program (SPMD); edge handling is pure
data:
  - zero-padded X_kv makes K=V=0 on out-of-range rows,
  - a host-provided validity column appended to V makes the softmax
    denominator (accumulated by the same AV matmul) skip those rows,
  - the window/ALiBi mask is applied as a multiplicative exp-mask after
    exp(): P = exp(S) * G, where G[h] is Toeplitz in (kv - q) and therefore
    shared by all query blocks (G = exp(-slope_h * |rel|) * [|rel| <= 256]);
    the host pre-unrolls it into the 6-chunk score layout.

Layout: scores are computed transposed (S^T[kv, q]) so the AV matmul needs
no on-chip transposes: lhsT = [V | valid], rhs = P^T gives O^T[d, q] plus
the denominator row.  All 6 kv-chunks of a q-block live in one [128,1536]
PSUM tile (3 banks; the even chunk of each bank carries start=True, which
zeroes the whole 2KB zero-region, so the odd chunk runs start=False), so
exp() and the mask multiply are single big ops.  Normalization: reciprocal
of the denominator row is partition-broadcast on GPSIMD (its only op - no
gpsimd library switching) and multiplied in on DVE.  The final projection
consumes O^T tiles directly as stationary operands.  All matmul operands
are float32r (full PE rate at moving dim >= 256).
"""

import os
import sys

if "/opt/trn_rl_repo" not in sys.path:
    sys.path.insert(0, "/opt/trn_rl_repo")

import numpy as np

import concourse.bacc as bacc
import concourse.mybir as mybir
import concourse.tile as tile
from concourse.bass_utils import run_bass_kernel_spmd

# ---------------------------------------------------------------- geometry
B, S, E = 2, 4096, 512
H, D = 8, 64
HD = H * D              # 512
HALF = 256              # window half-width (ATTENTION_WINDOW=512 -> 513 wide)
NCORES = 8
SQ = 4                  # sequence shards per batch
QROWS = S // SQ         # 1024 q rows per core
KVROWS = QROWS + 2 * HALF   # 1536 kv rows per core (with halo)
QB = 4                  # q blocks per core
QBLK = QROWS // QB      # 256 q cols per block
NCH = 6                 # kv chunks per q block
CBLK = 128              # kv chunk rows
SP6 = NCH * QBLK        # 1536: all chunks of a q block side by side

F32 = mybir.dt.float32
F32R = mybir.dt.float32r
BF16 = mybir.dt.bfloat16
PV_DT = F32R            # dtype of exp/mask/P^T/V path (F32R or BF16)

PWORK_BUFS = int(os.environ.get("K_PWORK", "4"))
GSQRT = os.environ.get("K_GSQRT", "0") == "1"   # derive G_h by chained sqrt
VCOPY_ACT = os.environ.get("K_VCOPY", "act") == "act"
QKCOPY_DVE = os.environ.get("K_QKCOPY", "act") == "dve"
GMUL_POOL = int(os.environ.get("K_GMUL", "0"))  # every Nth mask-mul on Pool

_CACHE = {}


def _build_program(repeats=1):
    """Build + compile the SPMD program (cached per process).

    repeats > 1 re-runs the whole computation that many times (same inputs,
    same outputs) - used only for wall-clock HW timing by difference.
    """
    key = ("nc", repeats)
    if key in _CACHE:
        return _CACHE[key]

    nc = bacc.Bacc("TRN2", target_bir_lowering=False, debug=False,
                   enable_asserts=True)

    xq_d = nc.dram_tensor("xqT", [E, QROWS], F32R, kind="ExternalInput")
    xkv_d = nc.dram_tensor("xkvT", [E, KVROWS], F32R, kind="ExternalInput")
    wq_d = nc.dram_tensor("wq", [E, HD], F32R, kind="ExternalInput")
    wk_d = nc.dram_tensor("wk", [E, HD], F32R, kind="ExternalInput")
    wv_d = nc.dram_tensor("wv", [E, HD], F32R, kind="ExternalInput")
    wo_d = nc.dram_tensor("wo", [HD, E], F32R, kind="ExternalInput")
    g_d = nc.dram_tensor("gmask", [1 if GSQRT else H, 128, SP6],
                     PV_DT, kind="ExternalInput")
    val_d = nc.dram_tensor("validc", [128, KVROWS // CBLK], F32,
                           kind="ExternalInput")
    y_d = nc.dram_tensor("y", [QROWS, E], F32, kind="ExternalOutput")

    EXP = mybir.ActivationFunctionType.Exp

    with tile.TileContext(nc) as tc:
        with (
            tc.tile_pool(name="wts", bufs=1) as wts,
            tc.tile_pool(name="bigx", bufs=8) as bigx,
            tc.tile_pool(name="proj", bufs=1) as proj,
            tc.tile_pool(name="pwork", bufs=PWORK_BUFS) as pwork,
            tc.tile_pool(name="small", bufs=4) as small,
            tc.tile_pool(name="bigp", bufs=2, space="PSUM") as bigp,
            tc.tile_pool(name="otp", bufs=2, space="PSUM") as otp,
        ):
            # ---- load weights (E on partitions, 4 tiles each)
            def load4(dram, name, cols):
                ts = []
                for e in range(4):
                    t = wts.tile([128, cols], F32R, tag=f"{name}{e}",
                                 name=f"{name}{e}")
                    nc.sync.dma_start(t[:], dram.ap()[128 * e:128 * (e + 1), :])
                    ts.append(t)
                return ts

            wq_sb = load4(wq_d, "wq", HD)
            wk_sb = load4(wk_d, "wk", HD)
            wv_sb = load4(wv_d, "wv", HD)
            wo_sb = load4(wo_d, "wo", E)

            valid_sb = small.tile([128, KVROWS // CBLK], F32, tag="validc",
                                  name="validc")
            nc.sync.dma_start(valid_sb[:], val_d.ap()[:])
            ones8 = small.tile([128, H], F32, tag="ones8", name="ones8")
            nc.vector.memset(ones8[:], 1.0)

            # ---- persistent activation tiles
            qt_sb = [proj.tile([128, QROWS], F32R, tag=f"qt{t}", name=f"qt{t}")
                     for t in range(4)]
            kt_sb = [proj.tile([128, KVROWS], F32R, tag=f"kt{t}",
                               name=f"kt{t}") for t in range(4)]
            # V tiles: head h occupies cols [65h, 65h+64), col 65h+64 = valid
            v_sb = [proj.tile([128, 65 * H], PV_DT, tag=f"v{b}", name=f"v{b}")
                    for b in range(KVROWS // CBLK)]
            ot_sb = [proj.tile([128, QROWS], F32R, tag=f"ot{t}", name=f"ot{t}")
                     for t in range(4)]

            for rep in range(repeats):
                xq_sb = []
                for e in range(4):
                    t = bigx.tile([128, KVROWS], F32R, tag="bigx",
                                  name=f"xq{e}")
                    nc.sync.dma_start(t[:, :QROWS],
                                      xq_d.ap()[128 * e:128 * (e + 1), :])
                    xq_sb.append(t)
                xkv_sb = []
                for e in range(4):
                    t = bigx.tile([128, KVROWS], F32R, tag="bigx",
                                  name=f"xkv{e}")
                    nc.sync.dma_start(t[:],
                                      xkv_d.ap()[128 * e:128 * (e + 1), :])
                    xkv_sb.append(t)

                # ---- projections (descending t: attention starts at h=7,
                # which reads tile 3 first)
                for t in range(3, -1, -1):
                    for n in range(QROWS // 512):
                        ps = bigp.tile([128, SP6], F32, tag="bigp",
                                       name="psq")
                        for e in range(4):
                            nc.tensor.matmul(
                                ps[:, :512],
                                lhsT=wq_sb[e][:, 128 * t:128 * (t + 1)],
                                rhs=xq_sb[e][:, 512 * n:512 * (n + 1)],
                                start=(e == 0), stop=(e == 3))
                        if QKCOPY_DVE:
                            nc.vector.tensor_copy(
                                qt_sb[t][:, 512 * n:512 * (n + 1)],
                                ps[:, :512])
                        else:
                            nc.scalar.copy(
                                qt_sb[t][:, 512 * n:512 * (n + 1)],
                                ps[:, :512])

                for t in range(3, -1, -1):
                    for n in range(KVROWS // 512):
                        ps = bigp.tile([128, SP6], F32, tag="bigp",
                                       name="psk")
                        for e in range(4):
                            nc.tensor.matmul(
                                ps[:, :512],
                                lhsT=wk_sb[e][:, 128 * t:128 * (t + 1)],
                                rhs=xkv_sb[e][:, 512 * n:512 * (n + 1)],
                                start=(e == 0), stop=(e == 3))
                        if QKCOPY_DVE:
                            nc.vector.tensor_copy(
                                kt_sb[t][:, 512 * n:512 * (n + 1)],
                                ps[:, :512])
                        else:
                            nc.scalar.copy(
                                kt_sb[t][:, 512 * n:512 * (n + 1)],
                                ps[:, :512])

                for blk in range(KVROWS // CBLK):
                    ps = bigp.tile([128, SP6], F32, tag="bigp", name="psv")
                    for e in range(4):
                        nc.tensor.matmul(
                            ps[:, :512],
                            lhsT=xkv_sb[e][:, 128 * blk:128 * (blk + 1)],
                            rhs=wv_sb[e][:],
                            start=(e == 0), stop=(e == 3))
                    vv = v_sb[blk][:].rearrange("p (h c) -> p h c", c=65)
                    veng = nc.scalar if VCOPY_ACT else nc.vector
                    (veng.copy if VCOPY_ACT else nc.vector.tensor_copy)(
                        vv[:, :, 0:64],
                        ps[:, :512].rearrange("p (h c) -> p h c", c=64))
                    nc.vector.tensor_scalar_mul(
                        vv[:, :, 64], ones8[:],
                        valid_sb[:, blk:blk + 1])

                # ---- Toeplitz exp-masks (reuse bigx slots once proj done)
                g_sb = [None] * H
                if GSQRT:
                    # Ship only G_{H-1} (slope 2^-8) and derive the rest by
                    # squaring: G_{h} = G_{h+1}^2 (slopes double downward).
                    # Squaring underflows to 0 only where the true mask is
                    # negligibly small; Square lives in every ACT table.
                    for h in range(H - 1, -1, -1):
                        t = bigx.tile([128, KVROWS], PV_DT, tag="bigx",
                                      name=f"g{h}")
                        if h == H - 1:
                            nc.sync.dma_start(t[:, :SP6], g_d.ap()[0])
                        else:
                            nc.scalar.square(t[:, :SP6],
                                             g_sb[h + 1][:, :SP6])
                        g_sb[h] = t
                else:
                    for h in range(H):
                        t = bigx.tile([128, KVROWS], PV_DT, tag="bigx",
                                      name=f"g{h}")
                        nc.sync.dma_start(t[:, :SP6], g_d.ap()[h])
                        g_sb[h] = t

                # ---- attention (h descending: with GSQRT the deepest
                # chain element g0 is needed last; qb outer so the output
                # projection of a q block can start as soon as its last
                # head finishes)
                for qb in range(QB):
                    for h in range(H - 1, -1, -1):
                        th, ph = h // 2, 64 * (h % 2)
                        qs = qt_sb[th][ph:ph + 64, QBLK * qb:QBLK * (qb + 1)]
                        sp = bigp.tile([128, SP6], F32, tag="bigp", name="sp6")
                        for c in range(NCH):
                            k0 = QBLK * qb + CBLK * c
                            nc.tensor.matmul(
                                sp[:, QBLK * c:QBLK * (c + 1)],
                                lhsT=kt_sb[th][ph:ph + 64, k0:k0 + CBLK],
                                rhs=qs,
                                start=(c % 2 == 0), stop=True,
                                skip_group_check=(c % 2 == 1))
                        pe6 = pwork.tile([128, SP6], PV_DT, tag="pw",
                                         name="pe6")
                        nc.scalar.activation(pe6[:], sp[:], EXP)
                        pm6 = pwork.tile([128, SP6], PV_DT, tag="pw",
                                         name="pm6")
                        if GMUL_POOL and (h * QB + qb) % GMUL_POOL == 0:
                            nc.gpsimd.tensor_mul(pm6[:], pe6[:],
                                                 g_sb[h][:, :SP6])
                        else:
                            nc.vector.tensor_mul(pm6[:], pe6[:],
                                                 g_sb[h][:, :SP6])
                        ot = otp.tile([65, QBLK], F32, tag="otp", name="ot")
                        for c in range(NCH):
                            nc.tensor.matmul(
                                ot[:],
                                lhsT=v_sb[2 * qb + c][:, 65 * h:65 * h + 65],
                                rhs=pm6[:, QBLK * c:QBLK * (c + 1)],
                                start=(c == 0), stop=(c == NCH - 1))
                        rec = small.tile([1, QBLK], F32, tag="rec", name="rec")
                        nc.vector.reciprocal(rec[:], ot[64:65, :])
                        bc = pwork.tile([64, QBLK], F32, tag="bc", name="bc")
                        nc.gpsimd.partition_broadcast(bc[:], rec[:])
                        nc.vector.tensor_mul(
                            ot_sb[th][ph:ph + 64, QBLK * qb:QBLK * (qb + 1)],
                            ot[0:64, :], bc[:])

                # ---- output projection
                for yb in range(QROWS // 128):
                    yp = bigp.tile([128, SP6], F32, tag="bigp", name="yp")
                    for t in range(4):
                        nc.tensor.matmul(
                            yp[:, :512],
                            lhsT=ot_sb[t][:, 128 * yb:128 * (yb + 1)],
                            rhs=wo_sb[t][:],
                            start=(t == 0), stop=(t == 3))
                    ystage = pwork.tile([128, 512], F32, tag="ys",
                                        name="ystage")
                    nc.scalar.copy(ystage[:], yp[:, :512])
                    nc.sync.dma_start(y_d.ap()[128 * yb:128 * (yb + 1), :],
                                      ystage[:])



    nc.compile()
    _CACHE[key] = nc
    return nc


def build_in_maps(inputs_q, inputs_kv, w_q, w_k, w_v, w_o):
    """Host-side sharding: slice/transpose/pad per core + mask tensors."""
    inputs_q = np.asarray(inputs_q, np.float32)
    inputs_kv = np.asarray(inputs_kv, np.float32)
    np_pv = mybir.dt.np(PV_DT)

    wq = np.ascontiguousarray(np.asarray(w_q, np.float32) * 0.125)
    wk = np.ascontiguousarray(np.asarray(w_k, np.float32))
    wv = np.ascontiguousarray(np.asarray(w_v, np.float32))
    wo = np.ascontiguousarray(np.asarray(w_o, np.float32))

    # Toeplitz exp-mask, pre-unrolled into the 6-chunk score layout:
    # chunk c, kv row r, q col i -> rel = i - r - 128c + 256
    slopes = np.array([2.0 ** (-(i + 1)) for i in range(H)], np.float64)
    r = np.arange(128)[:, None]
    i = np.arange(QBLK)[None, :]
    nheads_g = 1 if GSQRT else H
    g = np.empty((nheads_g, 128, SP6), np_pv)
    for c in range(NCH):
        rel = i - r - 128 * c + 256
        band = (np.abs(rel) <= HALF)
        for h in range(nheads_g):
            s = slopes[H - 1] if GSQRT else slopes[h]
            g[h, :, QBLK * c:QBLK * (c + 1)] = (
                np.exp(-s * np.abs(rel)) * band).astype(np_pv)

    in_maps = []
    for c in range(NCORES):
        b, sq = divmod(c, SQ)
        g0 = QROWS * sq
        xq = np.ascontiguousarray(inputs_q[b, g0:g0 + QROWS, :].T)
        kvlo = g0 - HALF
        lo, hi = max(0, kvlo), min(S, g0 + QROWS + HALF)
        xkv = np.zeros((E, KVROWS), np.float32)
        xkv[:, lo - kvlo:hi - kvlo] = inputs_kv[b, lo:hi, :].T
        valid = np.zeros((KVROWS,), np.float32)
        valid[lo - kvlo:hi - kvlo] = 1.0
        validc = np.ascontiguousarray(valid.reshape(KVROWS // CBLK, CBLK).T)
        in_maps.append({
            "xqT": xq, "xkvT": xkv,
            "wq": wq, "wk": wk, "wv": wv, "wo": wo,
            "gmask": g, "validc": validc,
        })
    return in_maps


def assemble_output(results):
    out = np.empty((B, S, E), np.float32)
    for c in range(NCORES):
        b, sq = divmod(c, SQ)
        out[b, QROWS * sq:QROWS * (sq + 1), :] = results[c]["y"]
    return out


def kernel(inputs_q, inputs_kv, w_q, w_k, w_v, w_o):
    nc = _build_program()
    in_maps = build_in_maps(inputs_q, inputs_kv, w_q, w_k, w_v, w_o)
    res = run_bass_kernel_spmd(nc, in_maps, core_ids=list(range(NCORES)))
    return assemble_output(res.results)
